# revision 41
# baseline (speedup 1.0000x reference)
"""DCNAlignNet on 8 trn2 NeuronCores — fully on-device.

One (batch, frame) unit per core (8 units = 8 cores, data-parallel). Per core
a single Bass program runs the whole chain: cr-conv, 4x (offset-conv ->
bilinear sample -> DCN einsum), rec-conv. All arithmetic fp32 (bf16 anywhere
blows the 2e-2 gate ~130x error amplification through the 4-layer
offset-dependent sampling chain).

Convs: 9-tap shifted-view matmuls (PSUM accumulated) over padded DRAM images.
Bilinear: per (group, tap, pixel) one dma_gather block of the full 2x2x8ch
bilinear patch from an interleaved padded DRAM image
il[g, y', x', (s,r,c) + 32-elem pad] (64-elem / 256B blocks, elem_step=64,
single_packet=False, rotated over the 4 SWDGE queues = 4 Q7 core-pairs).
Floor is exact via round-to-nearest(int16(x - 0.5)) with clamp [0.75, 130]
(bilinear extrapolation tolerance absorbs the +-1 ties). Gather indices are
built directly in the SWDGE wrapped layout (idx[p,f] = list[f*16 + p%16],
replicated per 16-partition band) with 8 PE permutation matmuls per strip.
Corner weights are applied on DVE; the 4 corner terms transpose-accumulate
into PSUM on the PE (transpose is a matmul), yielding samp^T[(k,c), pix] per
group; the DCN einsum is 8 accumulating K=72 matmuls per 512-pixel chunk.

Measured: ~20.3 ms/run (vs 171.8 s baseline), rel err 1.96e-4.
"""
import numpy as np

import concourse.bass as bass
import concourse.mybir as mybir
import concourse.tile as tile
from concourse import library_config
from concourse.bass_utils import run_bass_kernel_spmd
from concourse.library_overlay import lower_extended_insts

F32 = mybir.dt.float32
I16 = mybir.dt.int16
A = mybir.AluOpType

DG, NF, K, KK = 8, 64, 3, 9
H = W = 128
HP = 132            # il padded dim (pad 2 each side)
CP = 130            # conv padded dim (pad 1 each side)
ILG = HP * HP * 64  # il elems per group
NC = 8

LAST_EXEC_NS = None
LAST_SCOPE_TIMES = None
DBG = False
TRACE = False
STAGES = 3
NRUNS = 200

_WSPLIT_N = [0]


def _legalize_waits(nc):
    import bass_rust as _br
    for fn in nc.m.functions:
        for bb in fn.blocks:
            insts = list(bb.instructions)
            out, changed = [], False
            for inst in insts:
                si = inst.sync_info
                waits = list(si.on_wait) if si and si.on_wait else []
                cap = 2 if type(inst).__name__ == "InstEventSemaphore" else 1
                if len(waits) > cap:
                    changed = True
                    excess, keep = waits[:-cap], waits[-cap:]
                    while excess:
                        chunk, excess = excess[:2], excess[2:]
                        _WSPLIT_N[0] += 1
                        ev = mybir.InstEventSemaphore(name=f"WSPLIT-{_WSPLIT_N[0]}")
                        ev.engine = inst.engine
                        ev.sync_info = _br.SyncInfo(on_wait=chunk, on_update=[])
                        out.append(ev)
                    si.on_wait = keep
                out.append(inst)
            if changed:
                bb.instructions = out


_PROG = None


def _build_program():
    global _PROG
    if _PROG is not None:
        return _PROG
    nc = bass.Bass(num_swdge_queues=4)

    # ---- inputs
    in_pad = nc.dram_tensor("in_pad", [128, CP, CP], F32, kind="ExternalInput")
    suppT_d = nc.dram_tensor("suppT", [128, H, 64], F32, kind="ExternalInput")
    crw_d = nc.dram_tensor("crw", [9, 128, 64], F32, kind="ExternalInput")
    crb_d = nc.dram_tensor("crb", [64, 1], F32, kind="ExternalInput")
    offw_d = nc.dram_tensor("offw", [4, 9, 64, 144], F32, kind="ExternalInput")
    offb_d = nc.dram_tensor("offb", [4, 128, 144], F32, kind="ExternalInput")
    dcw_d = nc.dram_tensor("dcw", [4, 72, 8, 64], F32, kind="ExternalInput")
    dcb_d = nc.dram_tensor("dcb", [4, 64, 1], F32, kind="ExternalInput")
    recw_d = nc.dram_tensor("recw", [9, 64, 3], F32, kind="ExternalInput")
    recb_d = nc.dram_tensor("recb", [3, 1], F32, kind="ExternalInput")
    gy_d = nc.dram_tensor("gy", [128, H, 9], F32, kind="ExternalInput")
    gx_d = nc.dram_tensor("gx", [128, 9], F32, kind="ExternalInput")
    perm_d = nc.dram_tensor("perm", [8, 128, 128], F32, kind="ExternalInput")
    ident_d = nc.dram_tensor("ident", [128, 128], F32, kind="ExternalInput")

    # ---- scratch / outputs
    fea_a = nc.dram_tensor("fea_a", [64, CP, CP], F32, kind="Internal")
    fea_b = nc.dram_tensor("fea_b", [64, CP, CP], F32, kind="Internal")
    feaT_a = nc.dram_tensor("feaT_a", [128, H, 64], F32, kind="Internal")
    feaT_b = nc.dram_tensor("feaT_b", [128, H, 64], F32, kind="Internal")
    il_a = nc.dram_tensor("il_a", [8, HP, HP, 64], F32, kind="Internal")
    il_b = nc.dram_tensor("il_b", [8, HP, HP, 64], F32, kind="Internal")
    il_s = nc.dram_tensor("il_s", [8, HP, HP, 64], F32, kind="Internal")
    out_d = nc.dram_tensor("out", [3, H, W], F32, kind="ExternalOutput")
    if DBG:
        dbg_fea0 = nc.dram_tensor("dbg_fea0", [64, CP, CP], F32,
                                  kind="ExternalOutput")
        dbg_idxv = nc.dram_tensor("dbg_idxv", [128, 8, 16, 9], F32,
                                  kind="ExternalOutput")
        dbg_wrap = nc.dram_tensor("dbg_wrap", [128, 8, 1152], I16,
                                  kind="ExternalOutput")
        dbg_g = nc.dram_tensor("dbg_g", [128, 36, 64], F32,
                               kind="ExternalOutput")
        dbg_samp = nc.dram_tensor("dbg_samp", [72, 512], F32,
                                  kind="ExternalOutput")
        dbg_fea1 = nc.dram_tensor("dbg_fea1", [64, CP, CP], F32,
                                  kind="ExternalOutput")

    from contextlib import ExitStack
    with tile.TileContext(nc) as tc:
        with ExitStack() as _es:
            def _pool(name, bufs, space=None):
                kw = {"space": space} if space else {}
                return _es.enter_context(
                    tc.tile_pool(name=name, bufs=bufs, **kw))
            wp = _pool("wpool", 1)       # persistent weights
            stp = _pool("strip", 1)      # conv input strips
            osp = _pool("offsb", 1)
            otp = _pool("offT", 1)
            tap = _pool("tmpA", 2)       # index-math temps
            chp = _pool("chain", 1)      # per-axis y0/w/w1
            wcp = _pool("wcor", 1)       # corner weights
            ivp = _pool("idxv", 1)
            wrp = _pool("wrap", 1)
            gbp = _pool("gbuf", 4)
            hbp = _pool("hbuf", 1)
            ssp = _pool("sampsb", 1)
            fsp = _pool("feasb", 2)
            smp = _pool("small", 1)
            pcp = _pool("ps_conv", 1, "PSUM")
            pcbp = _pool("ps_convB", 1, "PSUM")
            potp = _pool("ps_offT", 1, "PSUM")
            pwp = _pool("ps_wrap", 1, "PSUM")
            psp = _pool("ps_samp", 2, "PSUM")
            pfp = _pool("ps_fea", 1, "PSUM")
            ptp = _pool("ps_feaT", 1, "PSUM")
            nc.gpsimd.load_library(library_config.mlp)
            nidx_reg = nc.gpsimd.to_reg(4608)

            # ---------- persistent weight loads ----------
            crw = wp.tile([128, 9, 64], F32)
            nc.sync.dma_start(crw[:], crw_d[:].rearrange("t p m -> p t m"))
            crb = wp.tile([64, 1], F32)
            nc.sync.dma_start(crb[:], crb_d[:])
            offw = wp.tile([64, 4, 9, 144], F32)
            nc.sync.dma_start(offw[:], offw_d[:].rearrange("l t p m -> p l t m"))
            offb = wp.tile([128, 4, 144], F32)
            nc.sync.dma_start(offb[:], offb_d[:].rearrange("l p m -> p l m"))
            dcw = wp.tile([72, 4, 8, 64], F32)
            nc.sync.dma_start(dcw[:], dcw_d[:].rearrange("l p g m -> p l g m"))
            dcb = wp.tile([64, 4, 1], F32)
            nc.sync.dma_start(dcb[:], dcb_d[:].rearrange("l p m -> p l m"))
            recw = wp.tile([64, 9, 3], F32)
            nc.sync.dma_start(recw[:], recw_d[:].rearrange("t p m -> p t m"))
            recb = wp.tile([3, 1], F32)
            nc.sync.dma_start(recb[:], recb_d[:])
            gx = wp.tile([128, 9], F32)
            nc.sync.dma_start(gx[:], gx_d[:])
            perm = wp.tile([128, 8, 128], F32)
            nc.sync.dma_start(perm[:], perm_d[:].rearrange("d p m -> p d m"))
            ident = wp.tile([128, 128], F32)
            nc.sync.dma_start(ident[:], ident_d[:])

            # ---------- zero il buffers ----------
            z = wp.tile([128, 1089], F32)
            nc.vector.memset(z[:], 0.0)
            for il in (il_a, il_b, il_s):
                flat = il[:].rearrange("g y x c -> g (y x c)")
                for g in range(8):
                    part = flat[g].rearrange("(p n) -> p n", p=128)
                    for e in range(8):
                        nc.sync.dma_start(part[:, e * 1089:(e + 1) * 1089], z[:])
            zf = z[0:64, :]
            for fd in (fea_a, fea_b):
                flat = fd[:].rearrange("c y x -> c (y x)")
                for e in range(15):
                    nc.sync.dma_start(flat[:, e * 1089:(e + 1) * 1089], zf)
                nc.sync.dma_start(flat[:, 15 * 1089:16900], zf[:, :565])

            # ---------- helpers ----------
            def write_il(il, feaT_src):
                # il[g, y', x', s*16+r*8+c] = fea[y'-2+r, x'-2+s, (g,c)]
                for g in range(8):
                    for r in range(2):
                        for s in range(2):
                            src = feaT_src[:, :, g * 8:(g + 1) * 8]
                            dst = il[g, 2 - r:CP - r, 2 - s:CP - s,
                                     s * 16 + r * 8: s * 16 + r * 8 + 8]
                            nc.sync.dma_start(dst.transpose([1, 0, 2]), src)

            def fea_tail(fea_sb, y0, fea_out, feaT_out):
                # fea_sb [64, 4, 128] strip -> fea_out interior; + feaT strip
                nc.sync.dma_start(
                    fea_out[:, 1 + y0:1 + y0 + 4, 1:129], fea_sb[:])
                if feaT_out is None:
                    return
                ftp = ptp.tile([128, 4, 64], F32)
                for yr in range(4):
                    nc.tensor.matmul(ftp[:, yr, :], fea_sb[:, yr, :],
                                     ident[:64, :64], start=True, stop=True,
                                     is_transpose=True)
                fts = smp.tile([128, 4, 64], F32, tag="feaTs")
                nc.scalar.copy(fts[:], ftp[:])
                nc.sync.dma_start(feaT_out[:, y0:y0 + 4, :], fts[:])

            # ---------- stage 0: cr conv ----------
            for s4 in range(32 if STAGES >= 0 else 0):
                y0 = s4 * 4
                ins = stp.tile([128, 6, CP], F32, tag="feastrip")
                nc.sync.dma_start(ins[:], in_pad[:, y0:y0 + 6, :])
                cp = pcp.tile([128, 512], F32, tag="convA")
                cpv = cp[0:64].rearrange("p (a b) -> p a b", a=4)
                for t in range(9):
                    ky, kx = t // 3, t % 3
                    nc.tensor.matmul(
                        cpv, crw[:, t, :], ins[:, ky:ky + 4, kx:kx + 128],
                        start=(t == 0), stop=(t == 8))
                fsb = fsp.tile([64, 4, 128], F32, tag="feasb")
                nc.vector.tensor_scalar(
                    fsb[:].rearrange("p a b -> p (a b)"),
                    cp[0:64], crb[:], None, A.add)
                fea_tail(fsb, y0, fea_a, feaT_a)
            write_il(il_a, feaT_a)
            write_il(il_s, suppT_d)
            if DBG:
                nc.sync.dma_start(dbg_fea0[:], fea_a[:])

            # ---------- 4 DCN layers ----------
            for li in range(4 if STAGES >= 1 else 0):
                fea_in = (fea_a, fea_b, fea_a, fea_b)[li]
                fea_out = (fea_b, fea_a, fea_b, fea_a)[li]
                il_in = (il_a, il_b, il_s, il_a)[li]
                il_out = (il_b, None, il_a, None)[li]
                feaT_out = (feaT_b, None, feaT_a, None)[li]

                for s16 in range(8):
                    yb = s16 * 16
                    fst = stp.tile([64, 18, CP], F32, tag="feastrip")
                    nc.sync.dma_start(fst[:], fea_in[:, yb:yb + 18, :])
                    offA = osp.tile([128, 4, 512], F32, tag="offA")
                    offB = osp.tile([16, 4, 512], F32, tag="offB")
                    for c4 in range(4):
                        pa = pcp.tile([128, 512], F32, tag="convA")
                        pb = pcbp.tile([16, 512], F32, tag="convB")
                        pav = pa[:].rearrange("p (a b) -> p a b", a=4)
                        pbv = pb[:].rearrange("p (a b) -> p a b", a=4)
                        for t in range(9):
                            ky, kx = t // 3, t % 3
                            rhs = fst[:, 4 * c4 + ky:4 * c4 + ky + 4,
                                      kx:kx + 128]
                            nc.tensor.matmul(pav, offw[:, li, t, 0:128], rhs,
                                             start=(t == 0), stop=(t == 8))
                            nc.tensor.matmul(pbv, offw[:, li, t, 128:144], rhs,
                                             start=(t == 0), stop=(t == 8))
                        nc.scalar.copy(offA[:, c4, :], pa[:])
                        nc.scalar.copy(offB[:, c4, :], pb[:])
                    # offT: transpose to [x, y16, 144] + bias
                    offT = otp.tile([128, 16, 144], F32, tag="offT")
                    for j in range(8):  # pairs of rows
                        pt = potp.tile([128, 2, 144], F32, tag="offTp")
                        for jj in range(2):
                            yr = j * 2 + jj
                            c4, yc = yr // 4, yr % 4
                            nc.tensor.matmul(
                                pt[:, jj, 0:128],
                                offA[:, c4, yc * 128:(yc + 1) * 128],
                                ident[:], start=True, stop=True,
                                is_transpose=True)
                            nc.tensor.matmul(
                                pt[:, jj, 128:144],
                                offB[:, c4, yc * 128:(yc + 1) * 128],
                                ident[:16, :16], start=True, stop=True,
                                is_transpose=True)
                        nc.vector.tensor_tensor(
                            offT[:, 2 * j:2 * j + 2, :], pt[:],
                            offb[:, li, :].unsqueeze(1).to_broadcast(
                                [128, 2, 144]),
                            A.add)

                    # ---- A: index math  [128, (g8, y16, k9)]
                    shp = [128, 8, 16, 9]
                    offv = offT[:].rearrange("x y (g k a) -> x a g y k",
                                             g=8, k=9)
                    gys = smp.tile([128, 16, 9], F32, tag="gys")
                    nc.sync.dma_start(gys[:], gy_d[:, yb:yb + 16, :])

                    def axis_chain(av, grid_ap):
                        # returns chain tile with [:,0]=floor, [:,1]=frac,
                        # [:,2]=1-frac
                        ch = chp.tile([128, 3, 8, 16, 9], F32,
                                      tag=f"chain{av}")
                        qf, wf, w1 = ch[:, 0], ch[:, 1], ch[:, 2]
                        p = tap.tile(shp, F32, tag="tmpA")
                        nc.vector.tensor_tensor(p[:], offv[:, av], grid_ap,
                                                A.add)
                        pc = tap.tile(shp, F32, tag="tmpA")
                        nc.vector.tensor_scalar(pc[:], p[:], 0.75, 130.0,
                                                A.max, A.min)
                        qi = tap.tile(shp, I16, tag="tmpAi")
                        nc.vector.tensor_scalar_sub(qi[:], pc[:], 0.5)
                        nc.vector.tensor_copy(qf, qi[:])
                        nc.vector.tensor_tensor(wf, pc[:], qf, A.subtract)
                        nc.vector.tensor_scalar(w1, wf, -1.0, 1.0,
                                                A.mult, A.add)
                        return qf, wf, w1

                    y0f, wy, wy1 = axis_chain(
                        0, gys[:].unsqueeze(1).to_broadcast(shp))
                    x0f, wx, wx1 = axis_chain(
                        1, gx[:].unsqueeze(1).unsqueeze(2).to_broadcast(shp))
                    wc = wcp.tile([128, 4, 8, 16, 9], F32, tag="wc")
                    for si, (wa, wb) in enumerate(
                            ((wy1, wx1), (wy1, wx), (wy, wx1), (wy, wx))):
                        nc.vector.tensor_tensor(wc[:, si], wa, wb, A.mult)
                    idxv = ivp.tile(shp, F32, tag="idxv")
                    nc.vector.scalar_tensor_tensor(
                        idxv[:], y0f, 132.0, x0f, A.mult, A.add)
                    if DBG and li == 0 and s16 == 0:
                        nc.sync.dma_start(dbg_idxv[:], idxv[:])

                    # ---- B: wrapped index build
                    wrap = wrp.tile([128, 8, 1152], I16, tag="wrap")
                    for d in range(8):
                        for g in range(8):
                            pw = pwp.tile([128, 144], F32, tag="wrapP")
                            nc.tensor.matmul(
                                pw[:], perm[:, d, :],
                                idxv[:, g].rearrange("x y k -> x (y k)"),
                                start=True, stop=True)
                            nc.vector.tensor_copy(wrap[:, g, d::8], pw[:])
                    if DBG and li == 0 and s16 == 0:
                        nc.sync.dma_start(dbg_wrap[:], wrap[:])

                    # ---- C: gather / corner-mult / transpose-acc / einsum
                    for sub in range(4 if STAGES >= 2 else 0):
                        yq = yb + sub * 4
                        pf = pfp.tile([64, 512], F32, tag="feaP")
                        for g in range(8 if STAGES >= 3 else 1):
                            gt = gbp.tile([128, 36, 64], F32, tag="G")
                            src_ap = bass.AP(il_in, g * ILG,
                                             [[64, HP * HP], [1, 64]])
                            nc.gpsimd.dma_gather(
                                out_ap=gt[:], in_ap=src_ap,
                                idxs_ap=wrap[:, g,
                                             sub * 288:(sub + 1) * 288],
                                num_idxs=4608, num_idxs_reg=nidx_reg,
                                elem_size=64, elem_step=64,
                                single_packet=False,
                                queue_num=(sub * 8 + g) % 4)
                            if DBG and li == 0 and s16 == 0 and sub == 0 \
                                    and g == 0:
                                nc.sync.dma_start(
                                    dbg_g[:],
                                    gt[:].rearrange("p a b -> p (a b)"))
                            ht = hbp.tile([128, 4, 36, 8], F32, tag="H")
                            for si in range(4):
                                s, r = si % 2, si // 2
                                nc.vector.tensor_tensor(
                                    ht[:, si], gt[:, :, s * 16 + r * 8:
                                                  s * 16 + r * 8 + 8],
                                    wc[:, si, g, 4 * sub:4 * sub + 4, :]
                                    .rearrange("x y k -> x (y k)")
                                    .unsqueeze(2).to_broadcast([128, 36, 8]),
                                    A.mult)
                            ps = psp.tile([72, 512], F32, tag="sampP")
                            for yr in range(4):
                                for si in range(4):
                                    nc.tensor.matmul(
                                        ps[:, yr * 128:(yr + 1) * 128],
                                        ht[:, si, yr * 9:(yr + 1) * 9, :]
                                        .rearrange("x k c -> x (k c)"),
                                        ident[:], start=(si == 0),
                                        stop=(si == 3), is_transpose=True)
                            ss = ssp.tile([72, 512], F32, tag="sampsb")
                            nc.scalar.copy(ss[:], ps[:])
                            if DBG and li == 0 and s16 == 0 and sub == 0 \
                                    and g == 0:
                                nc.sync.dma_start(dbg_samp[:], ss[:])
                            nc.tensor.matmul(pf[:], dcw[:, li, g, :], ss[:],
                                             start=(g == 0), stop=(g == 7),
                                             skip_group_check=True)
                        fsb = fsp.tile([64, 4, 128], F32, tag="feasb")
                        nc.vector.tensor_scalar(
                            fsb[:].rearrange("p a b -> p (a b)"),
                            pf[:], dcb[:, li, :], None, A.add)
                        fea_tail(fsb, yq, fea_out,
                                 feaT_out if il_out is not None else None)
                if il_out is not None:
                    write_il(il_out, feaT_out)
                if DBG and li == 0:
                    nc.sync.dma_start(dbg_fea1[:], fea_out[:])

            # ---------- rec conv ----------
            for s4 in range(32):
                y0 = s4 * 4
                fst = stp.tile([64, 18, CP], F32, tag="feastrip")
                nc.sync.dma_start(fst[:, 0:6, :], fea_a[:, y0:y0 + 6, :])
                rp = pfp.tile([3, 512], F32, tag="feaP")
                rpv = rp[:].rearrange("p (a b) -> p a b", a=4)
                for t in range(9):
                    ky, kx = t // 3, t % 3
                    nc.tensor.matmul(
                        rpv, recw[:, t, :], fst[:, ky:ky + 4, kx:kx + 128],
                        start=(t == 0), stop=(t == 8))
                osb = smp.tile([3, 4, 128], F32, tag="outsb")
                nc.vector.tensor_scalar(
                    osb[:].rearrange("p a b -> p (a b)"),
                    rp[:], recb[:], None, A.add)
                nc.sync.dma_start(out_d[:, y0:y0 + 4, :], osb[:])

    _legalize_waits(nc)
    lower_extended_insts(nc)
    _PROG = nc
    return nc


# ================= host side =================

def _prep_shared(cr_w, off_ws, dc_ws, rec_w, off_bs, dc_bs, cr_b, rec_b):
    t = {}
    t["crw"] = np.ascontiguousarray(
        cr_w.transpose(2, 3, 1, 0).reshape(9, 128, 64), np.float32)
    t["crb"] = cr_b.reshape(64, 1).astype(np.float32)
    offw = np.stack([w.transpose(2, 3, 1, 0).reshape(9, 64, 144)
                     for w in off_ws])
    t["offw"] = np.ascontiguousarray(offw, np.float32)
    t["offb"] = np.ascontiguousarray(
        np.stack([np.broadcast_to(b, (128, 144)) for b in off_bs]), np.float32)
    # dcw[l, (k*8+c), g, o] = w[o, g*8+c, ky, kx]
    dcw = np.empty((4, 72, 8, 64), np.float32)
    for l, w in enumerate(dc_ws):
        wr = w.reshape(64, 8, 8, 3, 3).transpose(3, 4, 2, 1, 0)  # ky kx c g o
        dcw[l] = wr.reshape(72, 8, 64)
    t["dcw"] = dcw
    t["dcb"] = np.stack([b.reshape(64, 1) for b in dc_bs]).astype(np.float32)
    t["recw"] = np.ascontiguousarray(
        rec_w.transpose(2, 3, 1, 0).reshape(9, 64, 3), np.float32)
    t["recb"] = rec_b.reshape(3, 1).astype(np.float32)
    ky = (np.arange(9) // 3 - 1).astype(np.float32)
    kx = (np.arange(9) % 3 - 1).astype(np.float32)
    gy = np.arange(H, dtype=np.float32)[:, None] + ky[None, :] + 2.0
    t["gy"] = np.broadcast_to(gy, (128, H, 9)).astype(np.float32).copy()
    gxv = np.arange(W, dtype=np.float32)[:, None] + kx[None, :] + 2.0
    t["gx"] = gxv.astype(np.float32)
    perm = np.zeros((8, 128, 128), np.float32)
    for d in range(8):
        for p in range(128):
            perm[d, d * 16 + p % 16, p] = 1.0
    t["perm"] = perm
    t["ident"] = np.eye(128, dtype=np.float32)
    return t


def _prep_unit(ref, supp):
    u = {}
    ip = np.zeros((128, CP, CP), np.float32)
    ip[0:64, 1:129, 1:129] = ref
    ip[64:128, 1:129, 1:129] = supp
    u["in_pad"] = ip
    u["suppT"] = np.ascontiguousarray(
        supp.transpose(2, 1, 0), np.float32)  # [x, y, c]
    return u


def kernel(precomputed_features, x_center, cr_w, cr_b, off1_w, off1_b,
           dc1_w, dc1_b, off2_w, off2_b, dc2_w, dc2_b, off3_w, off3_b,
           dc3_w, dc3_b, off4_w, off4_b, dc4_w, dc4_b, rec_w, rec_b):
    global LAST_EXEC_NS, LAST_SCOPE_TIMES
    import time
    pf = np.asarray(precomputed_features, np.float32)
    B, N = pf.shape[:2]
    center = N // 2
    frames = [i for i in range(N) if i != center]
    units = [(b, i) for b in range(B) for i in frames]
    assert len(units) == NC

    shared = _prep_shared(
        np.asarray(cr_w, np.float32),
        [np.asarray(w, np.float32) for w in (off1_w, off2_w, off3_w, off4_w)],
        [np.asarray(w, np.float32) for w in (dc1_w, dc2_w, dc3_w, dc4_w)],
        np.asarray(rec_w, np.float32),
        [np.asarray(b, np.float32) for b in (off1_b, off2_b, off3_b, off4_b)],
        [np.asarray(b, np.float32) for b in (dc1_b, dc2_b, dc3_b, dc4_b)],
        np.asarray(cr_b, np.float32), np.asarray(rec_b, np.float32))

    nc = _build_program()
    in_maps = []
    for b, i in units:
        u = _prep_unit(pf[b, center], pf[b, i])
        u.update(shared)
        in_maps.append(u)

    outs, exec_ns = _run_timed(nc, in_maps)
    LAST_EXEC_NS = exec_ns

    result = np.empty((B, N, 3, H, W), np.float32)
    result[:, center] = np.asarray(x_center, np.float32)
    for (b, i), r in zip(units, outs):
        result[b, i] = r["out"]
    return result


def _run_timed(nc, in_maps):
    """Mirror bass2jax.run_bass_via_pjrt's multi-core path, but stage all
    inputs on the devices first (no donation) and time repeated executions
    so the reported ns reflects device execution, not host->device staging."""
    import time

    import jax
    from jax.sharding import Mesh, PartitionSpec
    from jax.experimental.shard_map import shard_map

    import concourse.bass2jax as b2j

    b2j.install_neuronx_cc_hook()
    partition_name = (nc.partition_id_tensor.name
                      if nc.partition_id_tensor else None)
    in_names, out_names, out_avals, zero_outs = [], [], [], []
    for alloc in nc.m.functions[0].allocations:
        if not isinstance(alloc, mybir.MemoryLocationSet):
            continue
        name = alloc.memorylocations[0].name
        if alloc.kind == "ExternalInput":
            if name != partition_name:
                in_names.append(name)
        elif alloc.kind == "ExternalOutput":
            shape = tuple(alloc.tensor_shape)
            dtype = mybir.dt.np(alloc.dtype)
            out_names.append(name)
            out_avals.append(jax.core.ShapedArray(shape, dtype))
            zero_outs.append(np.zeros(shape, dtype))
    n_params = len(in_names)
    in_names.extend(out_names)
    if partition_name is not None:
        in_names.append(partition_name)

    def _body(*args):
        operands = list(args)
        if partition_name is not None:
            operands.append(b2j.partition_id_tensor())
        outs = b2j._bass_exec_p.bind(
            *operands,
            out_avals=tuple(out_avals),
            in_names=tuple(in_names),
            out_names=tuple(out_names),
            lowering_input_output_aliases=(),
            sim_require_finite=True,
            sim_require_nnan=True,
            nc=nc,
        )
        return tuple(outs)

    devices = jax.devices()[:NC]
    mesh = Mesh(np.asarray(devices), ("core",))
    nio = n_params + len(out_avals)
    sharded = jax.jit(
        shard_map(_body, mesh=mesh,
                  in_specs=(PartitionSpec("core"),) * nio,
                  out_specs=(PartitionSpec("core"),) * len(out_names),
                  check_rep=False),
        keep_unused=True)
    concat_in = [
        np.concatenate([np.asarray(m[in_names[i]]) for m in in_maps], axis=0)
        for i in range(n_params)
    ]
    concat_zeros = [
        np.zeros((NC * z.shape[0], *z.shape[1:]), z.dtype) for z in zero_outs
    ]
    sharding = jax.sharding.NamedSharding(mesh, PartitionSpec("core"))
    staged = [jax.device_put(a, sharding) for a in concat_in + concat_zeros]
    jax.block_until_ready(staged)

    out_arrs = sharded(*staged)       # warm-up (compiles + first run)
    jax.block_until_ready(out_arrs)
    # pipelined burst: amortizes the ~90ms axon dispatch round-trip so the
    # per-call figure approaches true device execution time
    k = max(1, NRUNS)
    t0 = time.time()
    rs = [sharded(*staged) for _ in range(k)]
    out_arrs = rs[-1]
    jax.block_until_ready(rs)
    exec_ns = int((time.time() - t0) / k * 1e9)
    outs = [
        {name: np.asarray(out_arrs[i]).reshape(NC, *out_avals[i].shape)[c]
         for i, name in enumerate(out_names)}
        for c in range(NC)
    ]
    return outs, exec_ns
